# revision 11
# baseline (speedup 1.0000x reference)
"""CropAndResize (TF-style, crop 14x14) on 8 Trainium2 NeuronCores.

Strategy (data-parallel over ROIs with 2-image load balancing):
  - Host: pair up the 8 images (largest box count with smallest) and give
    each pair to two cores; each core of a pair holds BOTH images of its
    pair in HBM and takes half the pair's boxes. This balances the per-core
    box count to ~n/8 (the int16 gather index reaches exactly 2 images).
  - Host builds a row-pair-interleaved bf16 image img2[y, x, :] =
    [img[y, x, :], img[min(y+1, H-1), x, :]] (channel-last). With that
    layout ALL FOUR bilinear corners of one output pixel are a single
    contiguous 2 KB span: cols (xs, xs+1) x rows (ti, ti+1) -> ONE SWDGE
    descriptor per output pixel, and bf16 halves the bytes.
  - Host computes the TF sampling grid bit-exactly in f32 and emits four
    combined corner weights per pixel (x-lerp * y-lerp * valid mask).
  - Device: per chunk of 8 boxes, two dma_gathers (896+768 descriptors)
    pull the corner quads (pixel on partitions, corners+channels on the
    free dim); the weighted 4-corner sum runs as one ACT scalar-mul plus a
    three-deep DVE scalar_tensor_tensor chain with per-partition scalar
    weights; the f32 result streams back to DRAM with one 13 KB contiguous
    descriptor per partition.
  - Host: scatter per-core outputs back to the original box order.
"""

import numpy as np
import ml_dtypes

import concourse.bacc as bacc
import concourse.bass as bass
import concourse.tile as tile
from concourse import mybir, library_config, bass_utils

H, W, C = 100, 152, 256
CROP = 14
PX = CROP * CROP          # 196 pixels per box
P = 128                   # SBUF partitions
NCORES = 8
CH = 8                    # boxes per chunk
QPAD = ((CH * PX + P - 1) // P) * P   # padded pixels per chunk (1664)
S = QPAD // P             # output slots per chunk (13)
NI = QPAD                 # gather descriptors per chunk (1 per pixel)
NPIX = H * W              # 15200 gatherable columns per image

F32 = mybir.dt.float32
BF16 = mybir.dt.bfloat16
I16 = mybir.dt.int16
MULT = mybir.AluOpType.mult
ADD = mybir.AluOpType.add

_cache = {}
LAST_EXEC_NS = None


def _grid_params(boxes):
    """Bit-exact f32 mirror of the reference sampling-grid math."""
    f = np.float32
    y1, x1, y2, x2 = boxes[:, 0], boxes[:, 1], boxes[:, 2], boxes[:, 3]
    h_scale = (y2 - y1) * f(H - 1) / f(CROP - 1)
    w_scale = (x2 - x1) * f(W - 1) / f(CROP - 1)
    ar = np.arange(CROP, dtype=np.float32)
    in_y = y1[:, None] * f(H - 1) + ar[None, :] * h_scale[:, None]
    in_x = x1[:, None] * f(W - 1) + ar[None, :] * w_scale[:, None]
    valid_y = (in_y >= 0) & (in_y <= H - 1)
    valid_x = (in_x >= 0) & (in_x <= W - 1)
    top = np.floor(in_y)
    left = np.floor(in_x)
    y_lerp = (in_y - top).astype(np.float32)
    x_lerp = (in_x - left).astype(np.float32)
    ti = np.clip(top, 0, H - 1).astype(np.int32)
    li = np.clip(left, 0, W - 1).astype(np.int32)
    ri = np.clip(left + 1, 0, W - 1).astype(np.int32)
    # column pair start + effective in-pair x lerp (li==ri only matters for
    # the valid in_x == W-1 edge, where col xs+1 is the wanted one)
    xs = np.minimum(li, W - 2).astype(np.int32)
    xw = np.where(li == ri, np.float32(1.0), x_lerp).astype(np.float32)
    return ti, y_lerp, xs, xw, valid_y, valid_x


def _build_core_inputs(boxes_k, img_sel_k):
    """Per-core gather indices + per-slot corner weights for m_pad boxes."""
    m_pad = boxes_k.shape[0]
    assert m_pad % CH == 0
    nch = m_pad // CH
    ti, yl, xs, xw, vy, vx = _grid_params(boxes_k)

    # per (box, i, j) flattened to q within each chunk
    desc = (img_sel_k[:, None, None] * NPIX
            + ti[:, :, None] * W + xs[:, None, :]).reshape(m_pad, PX)
    xw_q = np.broadcast_to(xw[:, None, :], (m_pad, CROP, CROP)).reshape(m_pad, PX)
    yw_q = np.broadcast_to(yl[:, :, None], (m_pad, CROP, CROP)).reshape(m_pad, PX)
    vm_q = (vy[:, :, None] & vx[:, None, :]).reshape(m_pad, PX).astype(np.float32)

    # combined corner weights, elem order [t0, b0, t1, b1]
    w4 = np.empty((m_pad, PX, 4), np.float32)
    w4[:, :, 0] = (1 - xw_q) * (1 - yw_q) * vm_q   # t0
    w4[:, :, 1] = (1 - xw_q) * yw_q * vm_q         # b0
    w4[:, :, 2] = xw_q * (1 - yw_q) * vm_q         # t1
    w4[:, :, 3] = xw_q * yw_q * vm_q               # b1

    idx_all = np.zeros((nch, NI), np.int16)
    w_all = np.zeros((nch, P, S * 4), np.float32)
    for ch in range(nch):
        sl = slice(ch * CH, (ch + 1) * CH)
        t = desc[sl].reshape(-1)
        idx_all[ch, : t.size] = t
        wq = np.zeros((QPAD, 4), np.float32)
        wq[: t.size] = w4[sl].reshape(-1, 4)
        # slot g, partition p <- q = g*128+p ; layout [P, S*4] = [p, g*4+c]
        w_all[ch] = wq.reshape(S, P, 4).transpose(1, 0, 2).reshape(P, S * 4)
    # wrapped idx layout: [16, NI//16] idx k at (k%16, k//16), tiled to 128
    wrapped = idx_all.reshape(nch, NI // 16, 16).transpose(0, 2, 1)
    idx_wrapped = np.tile(wrapped, (1, NCORES, 1))  # [nch, 128, NI//16]
    # preload layouts: [P, nch*NI//16] and [P, nch*S*4]
    idx_pre = idx_wrapped.transpose(1, 0, 2).reshape(P, nch * (NI // 16))
    wts_pre = w_all.transpose(1, 0, 2).reshape(P, nch * S * 4)
    return np.ascontiguousarray(idx_pre), np.ascontiguousarray(wts_pre)


def _build_program(nch):
    nc = bacc.Bacc("TRN2", target_bir_lowering=False, debug=False,
                   num_devices=NCORES, num_swdge_queues=2)
    img = nc.dram_tensor("img", [2 * NPIX * 2 * C], BF16, kind="ExternalInput")
    idx = nc.dram_tensor("idx", [P, nch * (NI // 16)], I16, kind="ExternalInput")
    wts = nc.dram_tensor("wts", [P, nch * S * 4], F32, kind="ExternalInput")
    out = nc.dram_tensor("out", [nch * P * S * C], F32, kind="ExternalOutput")

    # gather view: index unit = one 512-bf16 interleaved column, payload = 4C
    gather_src = bass.AP(img, 0, [(2 * C, 2 * NPIX - 1), (1, 4 * C)])

    with tile.TileContext(nc) as tc:
        with (
            tc.tile_pool(name="meta", bufs=1) as meta_pool,
            tc.tile_pool(name="gat", bufs=3) as gat_pool,
            tc.tile_pool(name="osb", bufs=3) as out_pool,
            tc.tile_pool(name="tmp", bufs=6) as tmp_pool,
        ):
            idx_t = meta_pool.tile([P, nch * (NI // 16)], I16, tag="idx")
            nc.sync.dma_start(idx_t[:], idx[:])
            w_t = meta_pool.tile([P, nch * S * 4], F32, tag="wts")
            nc.sync.dma_start(w_t[:], wts[:])
            nc.gpsimd.load_library(library_config.mlp)

            for ch in range(nch):
                g = gat_pool.tile([P, S, 4 * C], BF16, tag="g")
                # Each SWDGE queue's ring holds ~1024 descriptors; put the
                # chunk's two sub-gathers on separate queues so emission of
                # one never stalls on the other's ring draining.
                for qn, (j0, nj) in enumerate(((0, 896), (896, 768))):
                    nc.gpsimd.dma_gather(
                        g[:, j0 // P: (j0 + nj) // P, :], gather_src,
                        idx_t[:, ch * (NI // 16) + j0 // 16:
                              ch * (NI // 16) + (j0 + nj) // 16],
                        nj, nj, 4 * C, elem_step=2 * C, queue_num=qn)

                o = out_pool.tile([P, S, C], F32, tag="o")
                for sgi in range(S):
                    t0 = g[:, sgi, 0 * C:1 * C]
                    b0 = g[:, sgi, 1 * C:2 * C]
                    t1 = g[:, sgi, 2 * C:3 * C]
                    b1 = g[:, sgi, 3 * C:4 * C]
                    base = ch * S * 4 + sgi * 4
                    w_t0 = w_t[:, base + 0: base + 1]
                    w_b0 = w_t[:, base + 1: base + 2]
                    w_t1 = w_t[:, base + 2: base + 3]
                    w_b1 = w_t[:, base + 3: base + 4]

                    # out = w_t0*t0 + w_t1*t1 + w_b0*b0 + w_b1*b1
                    # Two schedules balance ACT/DVE/Pool occupancy: "tree"
                    # slots use 2 ACT muls + 2 DVE stt + 1 Pool tt; "chain"
                    # slots use 1 ACT mul + 3 DVE stt.
                    u = tmp_pool.tile([P, C], BF16, tag="u")
                    nc.scalar.mul(u[:], t0, w_t0)
                    a = tmp_pool.tile([P, C], BF16, tag="a")
                    nc.vector.scalar_tensor_tensor(a[:], t1, w_t1, u[:],
                                                   MULT, ADD)
                    if sgi % 4 != 3:  # tree (~3/4 of slots)
                        v = tmp_pool.tile([P, C], BF16, tag="v")
                        nc.scalar.mul(v[:], b0, w_b0)
                        b = tmp_pool.tile([P, C], BF16, tag="b")
                        nc.vector.scalar_tensor_tensor(b[:], b1, w_b1, v[:],
                                                       MULT, ADD)
                        nc.gpsimd.tensor_tensor(o[:, sgi, :], a[:], b[:], ADD)
                    else:  # chain
                        b = tmp_pool.tile([P, C], BF16, tag="b")
                        nc.vector.scalar_tensor_tensor(b[:], b0, w_b0, a[:],
                                                       MULT, ADD)
                        nc.vector.scalar_tensor_tensor(o[:, sgi, :], b1,
                                                       w_b1, b[:], MULT, ADD)

                out_ap = bass.AP(out, ch * P * S * C, [(S * C, P), (1, S * C)])
                nc.sync.dma_start(out_ap, o[:])

    nc.compile()
    return nc


def kernel(image, boxes, box_ind):
    image = np.asarray(image, dtype=np.float32)
    boxes = np.asarray(boxes, dtype=np.float32)
    box_ind = np.asarray(box_ind)
    n_boxes = boxes.shape[0]

    # pair images (largest count with smallest) and split each pair's boxes
    # between its two cores to balance per-core box counts
    sel = [np.where(box_ind == k)[0] for k in range(NCORES)]
    order = np.argsort([len(s) for s in sel])[::-1]
    pairs = [(int(order[i]), int(order[NCORES - 1 - i]))
             for i in range(NCORES // 2)]

    core_idx = []       # global box indices per core
    core_imgsel = []    # 0/1 image-within-pair per box
    for (a, b) in pairs:
        gidx = np.concatenate([sel[a], sel[b]])
        gsel = np.concatenate([np.zeros(len(sel[a]), np.int32),
                               np.ones(len(sel[b]), np.int32)])
        half = (len(gidx) + 1) // 2
        core_idx += [gidx[:half], gidx[half:]]
        core_imgsel += [gsel[:half], gsel[half:]]

    m_max = max(len(s) for s in core_idx)
    m_pad = ((m_max + CH - 1) // CH) * CH
    nch = m_pad // CH
    dummy = np.array([[0.25, 0.25, 0.75, 0.75]], np.float32)

    image_t = np.ascontiguousarray(image.transpose(0, 2, 3, 1))  # [B,H,W,C]
    # row-pair interleave + bf16: img2[y,x] = [img[y,x], img[min(y+1,H-1),x]]
    shifted = np.concatenate([image_t[:, 1:], image_t[:, -1:]], axis=1)
    img2 = np.concatenate([image_t, shifted], axis=-1)  # [B,H,W,2C]
    img2 = img2.astype(ml_dtypes.bfloat16)

    in_maps = []
    for t, (a, b) in enumerate(pairs):
        img_pair = np.concatenate([img2[a].reshape(-1), img2[b].reshape(-1)])
        for half in range(2):
            k = 2 * t + half
            bk = boxes[core_idx[k]]
            isel = core_imgsel[k]
            if bk.shape[0] < m_pad:
                npad = m_pad - bk.shape[0]
                bk = np.concatenate([bk, np.repeat(dummy, npad, 0)], axis=0)
                isel = np.concatenate([isel, np.zeros(npad, np.int32)])
            idx_pre, wts_pre = _build_core_inputs(bk, isel)
            in_maps.append({"img": img_pair, "idx": idx_pre, "wts": wts_pre})

    key = nch
    if key not in _cache:
        _cache[key] = _build_program(nch)
    nc = _cache[key]

    res = bass_utils.run_bass_kernel_spmd(nc, in_maps,
                                          core_ids=list(range(NCORES)))
    global LAST_EXEC_NS
    LAST_EXEC_NS = res.exec_time_ns

    out = np.zeros((n_boxes, C, CROP, CROP), np.float32)
    for k in range(NCORES):
        ok = res.results[k]["out"].reshape(nch, P, S, C)
        ok = ok.transpose(0, 2, 1, 3).reshape(nch, QPAD, C)[:, : CH * PX, :]
        ok = ok.reshape(m_pad, PX, C)[: len(core_idx[k])]
        out[core_idx[k]] = ok.transpose(0, 2, 1).reshape(-1, C, CROP, CROP)
    return out


# revision 14
# speedup vs baseline: 3.0778x; 3.0778x over previous
"""CropAndResize (TF-style, crop 14x14) on 8 Trainium2 NeuronCores.

Strategy (data-parallel over ROIs with 2-image load balancing):
  - Host: pair up the 8 images (largest box count with smallest) and give
    each pair to two cores; each core of a pair holds BOTH images of its
    pair in HBM and takes half the pair's boxes. This balances the per-core
    box count to ~n/8 (the int16 gather index reaches exactly 2 images).
  - Host builds a row-pair-interleaved bf16 image img2[y, x, :] =
    [img[y, x, :], img[min(y+1, H-1), x, :]] (channel-last). With that
    layout ALL FOUR bilinear corners of one output pixel are a single
    contiguous 2 KB span: cols (xs, xs+1) x rows (ti, ti+1) -> ONE SWDGE
    descriptor per output pixel, and bf16 halves the bytes.
  - Host computes the TF sampling grid bit-exactly in f32 and emits four
    combined corner weights per pixel (x-lerp * y-lerp * valid mask).
  - Device: per chunk of 8 boxes, two dma_gathers (896+768 descriptors)
    pull the corner quads (pixel on partitions, corners+channels on the
    free dim); the weighted 4-corner sum runs as one ACT scalar-mul plus a
    three-deep DVE scalar_tensor_tensor chain with per-partition scalar
    weights; the f32 result streams back to DRAM with one 13 KB contiguous
    descriptor per partition.
  - Host: scatter per-core outputs back to the original box order.
"""

import numpy as np
import ml_dtypes

import concourse.bacc as bacc
import concourse.bass as bass
import concourse.tile as tile
from concourse import mybir, library_config, bass_utils

H, W, C = 100, 152, 256
CROP = 14
PX = CROP * CROP          # 196 pixels per box
P = 128                   # SBUF partitions
NCORES = 8
CH = 8                    # boxes per chunk
QPAD = ((CH * PX + P - 1) // P) * P   # padded pixels per chunk (1664)
S = QPAD // P             # output slots per chunk (13)
NI = QPAD                 # gather descriptors per chunk (1 per pixel)
NPIX = H * W              # 15200 gatherable columns per image

F32 = mybir.dt.float32
BF16 = mybir.dt.bfloat16
I16 = mybir.dt.int16
MULT = mybir.AluOpType.mult
ADD = mybir.AluOpType.add

_cache = {}
LAST_EXEC_NS = None


def _grid_params(boxes):
    """Bit-exact f32 mirror of the reference sampling-grid math."""
    f = np.float32
    y1, x1, y2, x2 = boxes[:, 0], boxes[:, 1], boxes[:, 2], boxes[:, 3]
    h_scale = (y2 - y1) * f(H - 1) / f(CROP - 1)
    w_scale = (x2 - x1) * f(W - 1) / f(CROP - 1)
    ar = np.arange(CROP, dtype=np.float32)
    in_y = y1[:, None] * f(H - 1) + ar[None, :] * h_scale[:, None]
    in_x = x1[:, None] * f(W - 1) + ar[None, :] * w_scale[:, None]
    valid_y = (in_y >= 0) & (in_y <= H - 1)
    valid_x = (in_x >= 0) & (in_x <= W - 1)
    top = np.floor(in_y)
    left = np.floor(in_x)
    y_lerp = (in_y - top).astype(np.float32)
    x_lerp = (in_x - left).astype(np.float32)
    ti = np.clip(top, 0, H - 1).astype(np.int32)
    li = np.clip(left, 0, W - 1).astype(np.int32)
    ri = np.clip(left + 1, 0, W - 1).astype(np.int32)
    # column pair start + effective in-pair x lerp (li==ri only matters for
    # the valid in_x == W-1 edge, where col xs+1 is the wanted one)
    xs = np.minimum(li, W - 2).astype(np.int32)
    xw = np.where(li == ri, np.float32(1.0), x_lerp).astype(np.float32)
    return ti, y_lerp, xs, xw, valid_y, valid_x


def _build_core_inputs(boxes_k, img_sel_k):
    """Per-core gather indices + per-slot corner weights for m_pad boxes."""
    m_pad = boxes_k.shape[0]
    assert m_pad % CH == 0
    nch = m_pad // CH
    ti, yl, xs, xw, vy, vx = _grid_params(boxes_k)

    # per (box, i, j) flattened to q within each chunk
    desc = (img_sel_k[:, None, None] * NPIX
            + ti[:, :, None] * W + xs[:, None, :]).reshape(m_pad, PX)
    xw_q = np.broadcast_to(xw[:, None, :], (m_pad, CROP, CROP)).reshape(m_pad, PX)
    yw_q = np.broadcast_to(yl[:, :, None], (m_pad, CROP, CROP)).reshape(m_pad, PX)
    vm_q = (vy[:, :, None] & vx[:, None, :]).reshape(m_pad, PX).astype(np.float32)

    # combined corner weights, elem order [t0, b0, t1, b1]
    w4 = np.empty((m_pad, PX, 4), np.float32)
    w4[:, :, 0] = (1 - xw_q) * (1 - yw_q) * vm_q   # t0
    w4[:, :, 1] = (1 - xw_q) * yw_q * vm_q         # b0
    w4[:, :, 2] = xw_q * (1 - yw_q) * vm_q         # t1
    w4[:, :, 3] = xw_q * yw_q * vm_q               # b1

    idx_all = np.zeros((nch, NI), np.int16)
    w_all = np.zeros((nch, P, S * 4), np.float32)
    for ch in range(nch):
        sl = slice(ch * CH, (ch + 1) * CH)
        t = desc[sl].reshape(-1)
        idx_all[ch, : t.size] = t
        wq = np.zeros((QPAD, 4), np.float32)
        wq[: t.size] = w4[sl].reshape(-1, 4)
        # slot g, partition p <- q = g*128+p ; layout [P, S*4] = [p, g*4+c]
        w_all[ch] = wq.reshape(S, P, 4).transpose(1, 0, 2).reshape(P, S * 4)
    # wrapped idx layout: [16, NI//16] idx k at (k%16, k//16), tiled to 128
    wrapped = idx_all.reshape(nch, NI // 16, 16).transpose(0, 2, 1)
    idx_wrapped = np.tile(wrapped, (1, NCORES, 1))  # [nch, 128, NI//16]
    # preload layouts: [P, nch*NI//16] and [P, nch*S*4]
    idx_pre = idx_wrapped.transpose(1, 0, 2).reshape(P, nch * (NI // 16))
    wts_pre = w_all.transpose(1, 0, 2).reshape(P, nch * S * 4)
    return np.ascontiguousarray(idx_pre), np.ascontiguousarray(wts_pre)


def _build_program(nch):
    nc = bacc.Bacc("TRN2", target_bir_lowering=False, debug=False,
                   num_devices=NCORES)
    img = nc.dram_tensor("img", [2 * NPIX * 2 * C], BF16, kind="ExternalInput")
    idx = nc.dram_tensor("idx", [P, nch * (NI // 16)], I16, kind="ExternalInput")
    wts = nc.dram_tensor("wts", [P, nch * S * 4], F32, kind="ExternalInput")
    out = nc.dram_tensor("out", [nch * P * S * C], F32, kind="ExternalOutput")

    # gather view: index unit = one 512-bf16 interleaved column, payload = 4C
    gather_src = bass.AP(img, 0, [(2 * C, 2 * NPIX - 1), (1, 4 * C)])

    with tile.TileContext(nc) as tc:
        with (
            tc.tile_pool(name="meta", bufs=1) as meta_pool,
            tc.tile_pool(name="gat", bufs=3) as gat_pool,
            tc.tile_pool(name="osb", bufs=3) as out_pool,
            tc.tile_pool(name="tmp", bufs=6) as tmp_pool,
        ):
            idx_t = meta_pool.tile([P, nch * (NI // 16)], I16, tag="idx")
            nc.sync.dma_start(idx_t[:], idx[:])
            w_t = meta_pool.tile([P, nch * S * 4], F32, tag="wts")
            nc.sync.dma_start(w_t[:], wts[:])
            nc.gpsimd.load_library(library_config.mlp)

            for ch in range(nch):
                g = gat_pool.tile([P, S, 4 * C], BF16, tag="g")
                # SWDGE ring holds ~1024 descriptors; 512-desc sub-gathers
                # keep emission from stalling on ring drain.
                GU = 512
                for j0 in range(0, NI, GU):
                    nj = min(GU, NI - j0)
                    nc.gpsimd.dma_gather(
                        g[:, j0 // P: (j0 + nj) // P, :], gather_src,
                        idx_t[:, ch * (NI // 16) + j0 // 16:
                              ch * (NI // 16) + (j0 + nj) // 16],
                        nj, nj, 4 * C, elem_step=2 * C)

                o = out_pool.tile([P, S, C], F32, tag="o")
                for sgi in range(S):
                    t0 = g[:, sgi, 0 * C:1 * C]
                    b0 = g[:, sgi, 1 * C:2 * C]
                    t1 = g[:, sgi, 2 * C:3 * C]
                    b1 = g[:, sgi, 3 * C:4 * C]
                    base = ch * S * 4 + sgi * 4
                    w_t0 = w_t[:, base + 0: base + 1]
                    w_b0 = w_t[:, base + 1: base + 2]
                    w_t1 = w_t[:, base + 2: base + 3]
                    w_b1 = w_t[:, base + 3: base + 4]

                    # out = (w_t0*t0 + w_t1*t1) + (w_b0*b0 + w_b1*b1)
                    # tree schedule: 2 ACT muls + 2 DVE stt + 1 DVE tt
                    u = tmp_pool.tile([P, C], BF16, tag="u")
                    nc.scalar.mul(u[:], t0, w_t0)
                    a = tmp_pool.tile([P, C], BF16, tag="a")
                    nc.vector.scalar_tensor_tensor(a[:], t1, w_t1, u[:],
                                                   MULT, ADD)
                    v = tmp_pool.tile([P, C], BF16, tag="v")
                    nc.scalar.mul(v[:], b0, w_b0)
                    b = tmp_pool.tile([P, C], BF16, tag="b")
                    nc.vector.scalar_tensor_tensor(b[:], b1, w_b1, v[:],
                                                   MULT, ADD)
                    nc.vector.tensor_tensor(o[:, sgi, :], a[:], b[:], ADD)

                out_ap = bass.AP(out, ch * P * S * C, [(S * C, P), (1, S * C)])
                nc.sync.dma_start(out_ap, o[:])

    nc.compile()
    return nc


def kernel(image, boxes, box_ind):
    image = np.asarray(image, dtype=np.float32)
    boxes = np.asarray(boxes, dtype=np.float32)
    box_ind = np.asarray(box_ind)
    n_boxes = boxes.shape[0]

    # pair images (largest count with smallest) and split each pair's boxes
    # between its two cores to balance per-core box counts
    sel = [np.where(box_ind == k)[0] for k in range(NCORES)]
    order = np.argsort([len(s) for s in sel])[::-1]
    pairs = [(int(order[i]), int(order[NCORES - 1 - i]))
             for i in range(NCORES // 2)]

    core_idx = []       # global box indices per core
    core_imgsel = []    # 0/1 image-within-pair per box
    for (a, b) in pairs:
        gidx = np.concatenate([sel[a], sel[b]])
        gsel = np.concatenate([np.zeros(len(sel[a]), np.int32),
                               np.ones(len(sel[b]), np.int32)])
        half = (len(gidx) + 1) // 2
        core_idx += [gidx[:half], gidx[half:]]
        core_imgsel += [gsel[:half], gsel[half:]]

    m_max = max(len(s) for s in core_idx)
    m_pad = ((m_max + CH - 1) // CH) * CH
    nch = m_pad // CH
    dummy = np.array([[0.25, 0.25, 0.75, 0.75]], np.float32)

    image_t = np.ascontiguousarray(image.transpose(0, 2, 3, 1))  # [B,H,W,C]
    # row-pair interleave + bf16: img2[y,x] = [img[y,x], img[min(y+1,H-1),x]]
    shifted = np.concatenate([image_t[:, 1:], image_t[:, -1:]], axis=1)
    img2 = np.concatenate([image_t, shifted], axis=-1)  # [B,H,W,2C]
    img2 = img2.astype(ml_dtypes.bfloat16)

    in_maps = []
    for t, (a, b) in enumerate(pairs):
        img_pair = np.concatenate([img2[a].reshape(-1), img2[b].reshape(-1)])
        for half in range(2):
            k = 2 * t + half
            bk = boxes[core_idx[k]]
            isel = core_imgsel[k]
            if bk.shape[0] < m_pad:
                npad = m_pad - bk.shape[0]
                bk = np.concatenate([bk, np.repeat(dummy, npad, 0)], axis=0)
                isel = np.concatenate([isel, np.zeros(npad, np.int32)])
            idx_pre, wts_pre = _build_core_inputs(bk, isel)
            in_maps.append({"img": img_pair, "idx": idx_pre, "wts": wts_pre})

    key = nch
    if key not in _cache:
        _cache[key] = _build_program(nch)
    nc = _cache[key]

    res = bass_utils.run_bass_kernel_spmd(nc, in_maps,
                                          core_ids=list(range(NCORES)))
    global LAST_EXEC_NS
    LAST_EXEC_NS = res.exec_time_ns

    out = np.zeros((n_boxes, C, CROP, CROP), np.float32)
    for k in range(NCORES):
        ok = res.results[k]["out"].reshape(nch, P, S, C)
        ok = ok.transpose(0, 2, 1, 3).reshape(nch, QPAD, C)[:, : CH * PX, :]
        ok = ok.reshape(m_pad, PX, C)[: len(core_idx[k])]
        out[core_idx[k]] = ok.transpose(0, 2, 1).reshape(-1, C, CROP, CROP)
    return out
